# revision 49
# baseline (speedup 1.0000x reference)
"""Trainium2 Bass kernel for KeyframeSelectionNetwork.

Math (per (b, v) video of T=64 frames, F=1024 features):
  GCN with self-loops + one edge (frame0 -> frame1), symmetric norm:
    out[t] = x[t] @ W_gcn                      for t != 1
    out[1] = (0.5*x[1] + (1/sqrt(2))*x[0]) @ W_gcn
  pooled = max_t out[t] + b_gcn
  h = relu(pooled.reshape(B, V*F) @ W1 + b1)  -> [B, 256]
  key = sigmoid(h @ W2 + b2)                  -> [B, V, T]

Strategy: data-parallel over batch across 8 cores (8 videos' batches each).
Host-side prep (per core): X is pre-transposed to chunk-major X^T layout
([chunk, 128 f-part, KC, 512 nodes]) and cast to fp8-e4m3; W_gcn to fp8,
W1/W2 to bf16, all pre-arranged so every load is a single fat contiguous
HWDGE DMA (per-partition lines of 4-32KB).

Device per rep (~9.3 MB HBM traffic, ~70us, PE-bound at the fp8 roofline):
  - ~21 fat DMAs alternating the two HWDGE queues (sync/act); X chunks in
    k-halves so matmuls start on first-half arrival.
  - edge combine applied in-place on the fp8 X tile (DVE ops per k-half on
    [128, KC/2, 8] strided views: x1' = 0.5*x1 + rsqrt2*x0).
  - GCN matmul Y^T[fout, nodes] via fp8 DoubleRow PE matmuls (contracts
    2x128 K-rows per pass, 2x bf16 throughput), PSUM-accumulated.
  - max-pool over t via DVE reduce_max per PSUM tile -> pooledT [fout, b*v],
    then bias-add + bf16 cast per (j, chunk) on the idle gpsimd engine.
  - MLP in bf16: pooled slices as stationary, biases folded in as rank-1
    matmuls (ones.T @ b), relu/sigmoid on ACT engine.
  - PSUM pools are shared across reps (no per-rep pool barriers) and the
    per-rep outputs are max-combined into one [BL, OUT] store so the
    R-reps timing NEFF has the same host-visible output as R=1.
"""

import sys

sys.path.insert(0, "/opt/trn_rl_repo")

import numpy as np

B, V, T, F = 64, 8, 64, 1024
NCORES = 8
BL = B // NCORES  # batches per core (8)
NLOC = BL * V * T  # nodes per core (4096)
H1 = 256
OUT = V * T  # 512
P = 128
KC = F // P  # 8 contraction chunks
JC = F // P  # 8 output-feature chunks
CHUNK = V * T  # 512 nodes per chunk = one batch's videos
NCH = NLOC // CHUNK  # 8 chunks per core
RT2 = float(1.0 / np.sqrt(2.0))

CFG = dict(
    gcn_fp8=True,  # fp8 X/W_gcn with DoubleRow matmuls; else bf16
    combine_on_x=True,  # edge combine on X tile (else post-matmul in PSUM)
    x_bufs=8,
    w_bufs=2,
    psum_bufs=7,  # 7 x 1-bank yp tiles + 1 shared epilogue bank
    defer_epilogue=False,  # emitting MLP mid-next-rep measured slightly worse
    fused_bias_cast=True,  # bias+bf16-cast per (j,c) right after each reduce
)

_STATE = None


def _build_nc(cfg, reps=1):
    import concourse.bacc as bacc
    import concourse.tile as tile
    from concourse import mybir

    f32 = mybir.dt.float32
    bf16 = mybir.dt.bfloat16
    fp8 = mybir.dt.float8e4
    AF = mybir.ActivationFunctionType
    ALU = mybir.AluOpType
    gcn_dt = fp8 if cfg["gcn_fp8"] else bf16

    nc = bacc.Bacc(None, target_bir_lowering=False, debug=False)
    x_d = nc.dram_tensor("xt", [NCH * P, KC * CHUNK], gcn_dt, kind="ExternalInput")
    wg_d = nc.dram_tensor("wg", [P, KC * F], gcn_dt, kind="ExternalInput")
    w1_d = nc.dram_tensor("w1", [P, V * KC * H1], bf16, kind="ExternalInput")
    w2_d = nc.dram_tensor("w2", [P, 2 * OUT], bf16, kind="ExternalInput")
    bg_d = nc.dram_tensor("bg", [P, JC], f32, kind="ExternalInput")
    b1_d = nc.dram_tensor("b1", [1, H1], f32, kind="ExternalInput")
    b2_d = nc.dram_tensor("b2", [1, OUT], f32, kind="ExternalInput")
    id8_d = nc.dram_tensor("id8", [BL, BL], f32, kind="ExternalInput")
    # Single [BL, OUT] output for any reps: per-rep results are max-combined
    # on device (keeps every rep live against DCE) so the host-side output
    # fetch is identical for the R=1 and R=reps NEFFs and cancels in the
    # marginal-time measurement.
    out_d = nc.dram_tensor("out", [BL, OUT], f32, kind="ExternalOutput")

    with tile.TileContext(nc) as tc:
      with (
          tc.tile_pool(name="xpool", bufs=cfg["x_bufs"]) as xpool,
          tc.tile_pool(name="wpool", bufs=cfg["w_bufs"]) as wpool,
          tc.tile_pool(name="spool", bufs=2) as spool,
          tc.tile_pool(name="fpool", bufs=2) as fpool,
          tc.tile_pool(name="apool", bufs=1) as apool,
          tc.tile_pool(name="mpsum", bufs=cfg["psum_bufs"], space="PSUM") as mpsum,
          tc.tile_pool(name="lpsum", bufs=1, space="PSUM") as lpsum,
      ):
        oacc_sb = None
        if reps > 1:
            oacc_sb = apool.tile([BL, OUT], f32, tag="oacc", name="oacc_sb")
        # Software pipelining across reps: each rep's epilogue (MLP) is
        # emitted AFTER the next rep's first chunk of Y-matmuls, so the PE's
        # in-order queue never stalls waiting for the pooling reduces.
        pending_epi = [None]
        for _rep in range(reps):
            # ---- prologue DMAs (all fat contiguous loads) ----
            wg_sb = wpool.tile([P, KC, F], gcn_dt, tag="wg")
            nc.sync.dma_start(
                wg_sb[:], wg_d[:].rearrange("p (k f) -> p k f", f=F)
            )
            xts = {}

            def load_x(c):
                t_ = xpool.tile([P, KC, CHUNK], gcn_dt, tag="x")
                # alternate the two HWDGE queues (sync/act) for DMA
                # throughput; split in k-halves so the first matmuls of the
                # chunk only wait on the first half's arrival
                eng = nc.sync if c % 2 == 0 else nc.scalar
                src = x_d[c * P : (c + 1) * P, :].rearrange(
                    "p (k n) -> p k n", n=CHUNK
                )
                # single full-chunk DMA: with 8-deep prefetch the arrival
                # granularity of a k-half split no longer matters, and
                # halving the DMA instruction count cuts issue + semaphore
                # overhead on the sync/act queues
                eng.dma_start(t_[:], src)
                xts[c] = t_

            nx_pre = min(cfg["x_bufs"], NCH)
            for c in range(nx_pre):
                load_x(c)
            bg_sb = spool.tile([P, JC], f32, tag="bg")
            nc.sync.dma_start(bg_sb[:], bg_d[:])
            b1_sb = spool.tile([1, H1], f32, tag="b1")
            nc.sync.dma_start(b1_sb[:], b1_d[:])
            b2_sb = spool.tile([1, OUT], f32, tag="b2")
            nc.sync.dma_start(b2_sb[:], b2_d[:])
            id8_sb = spool.tile([BL, BL], f32, tag="id8")
            nc.sync.dma_start(id8_sb[:], id8_d[:])
            ones_sb = spool.tile([1, BL], f32, tag="ones")
            nc.gpsimd.memset(ones_sb[:], 1.0)
            w2_sb = wpool.tile([P, 2, OUT], bf16, tag="w2")
            nc.sync.dma_start(
                w2_sb[:], w2_d[:].rearrange("p (m n) -> p m n", n=OUT)
            )
            # w1 (4MB) is loaded in 8 x 512KB pieces interleaved with the
            # chunk loop (see below): a monolithic DMA here would sit on the
            # scalar queue ahead of the NEXT rep's odd-chunk X loads, which
            # are needed ~7us into the rep while w1 isn't needed until the
            # epilogue (priority inversion seen as early-rep PE gaps).
            w1_sb = wpool.tile([P, V * KC, H1], bf16, tag="w1")
            w1_src = w1_d[:].rearrange("p (i n) -> p i n", n=H1)

            pooledT = spool.tile([P, JC, NCH * V], f32, tag="pooledT")
            pooled_m = spool.tile([P, JC, NCH * V], bf16, tag="pooled_m")

            # ---- main loop: per chunk, combine fixup + JC matmul/pool ----
            if True:
                for c in range(NCH):
                    xt = xts.pop(c)
                    if cfg["combine_on_x"]:
                        # x1' = 0.5*x1 + rsqrt2*x0, in place on the fp8 tile
                        # (done per k-half to match the split DMA arrival)
                        x4 = xt[:].rearrange("p k (g t) -> p k g t", t=T)
                        for h in range(2):
                            ks = slice(h * KC // 2, (h + 1) * KC // 2)
                            col0 = x4[:, ks, :, 0]
                            col1 = x4[:, ks, :, 1]
                            nc.vector.tensor_scalar_mul(col1, col1, 0.5)
                            nc.vector.scalar_tensor_tensor(
                                col1, col0, RT2, col1, ALU.mult, ALU.add
                            )
                    for j in range(JC):
                        yp = mpsum.tile([P, CHUNK], f32, tag="yp")
                        for q in range(KC // 2):
                            nc.tensor.matmul(
                                yp[:],
                                wg_sb[:, 2 * q : 2 * q + 2, j * P : (j + 1) * P],
                                xt[:, 2 * q : 2 * q + 2, :],
                                start=(q == 0),
                                stop=(q == KC // 2 - 1),
                                perf_mode=mybir.MatmulPerfMode.DoubleRow,
                            )
                        nc.vector.reduce_max(
                            pooledT[:, j, c * V : (c + 1) * V],
                            yp[:].rearrange("p (g t) -> p g t", t=T),
                            axis=mybir.AxisListType.X,
                        )
                        if cfg["fused_bias_cast"]:
                            # bias-add + bf16 cast on the idle gpsimd engine
                            nc.gpsimd.tensor_scalar_add(
                                pooled_m[:, j, c * V : (c + 1) * V],
                                pooledT[:, j, c * V : (c + 1) * V],
                                bg_sb[:, j : j + 1],
                            )
                    if c == 0 and pending_epi[0] is not None:
                        pending_epi[0]()
                        pending_epi[0] = None
                    nc.scalar.dma_start(
                        w1_sb[:, c * V : (c + 1) * V, :],
                        w1_src[:, c * V : (c + 1) * V, :],
                    )
                    if c + nx_pre < NCH:
                        load_x(c + nx_pre)

            # ---- epilogue: bias (fused into bf16 cast), MLP; deferred
            # until after the next rep's first chunk (see pending_epi) ----
            def _epilogue(
                _rep=_rep,
                pooledT=pooledT,
                pooled_m=pooled_m,
                bg_sb=bg_sb,
                w1_sb=w1_sb,
                w2_sb=w2_sb,
                b1_sb=b1_sb,
                b2_sb=b2_sb,
                id8_sb=id8_sb,
                ones_sb=ones_sb,
            ):
                if not cfg["fused_bias_cast"]:
                    for j in range(JC):
                        nc.scalar.activation(
                            pooled_m[:, j, :],
                            pooledT[:, j, :],
                            AF.Identity,
                            bias=bg_sb[:, j : j + 1],
                        )

                # hp/thp/op share one lpsum slot (used strictly sequentially,
                # same tag) so the epilogue needs only 1 PSUM bank
                hp = lpsum.tile([BL, H1], f32, tag="lp", name="hp")
                for v in range(V):
                    for fc in range(KC):
                        i = v * KC + fc
                        lhs = pooled_m[:, fc, :].rearrange(
                            "p (b w) -> p w b", w=V
                        )[:, v, :]
                        nc.tensor.matmul(
                            hp[:], lhs, w1_sb[:, i, :], start=(i == 0), stop=False
                        )
                nc.tensor.matmul(hp[:], ones_sb[:], b1_sb[:], start=False, stop=True)
                h_sb = spool.tile([BL, H1], f32, tag="h")
                nc.scalar.activation(h_sb[:], hp[:], AF.Relu)

                ht_sb = spool.tile([P, 2, BL], bf16, tag="ht")
                for m in range(2):
                    thp = lpsum.tile([P, BL], f32, tag="lp", name="thp")
                    nc.tensor.transpose(
                        thp[:], h_sb[:, m * P : (m + 1) * P], id8_sb[:]
                    )
                    nc.vector.tensor_copy(ht_sb[:, m, :], thp[:])

                op = lpsum.tile([BL, OUT], f32, tag="lp", name="op")
                for m in range(2):
                    nc.tensor.matmul(
                        op[:], ht_sb[:, m, :], w2_sb[:, m, :], start=(m == 0),
                        stop=False,
                    )
                nc.tensor.matmul(op[:], ones_sb[:], b2_sb[:], start=False, stop=True)
                o_sb = spool.tile([BL, OUT], f32, tag="o")
                nc.scalar.activation(o_sb[:], op[:], AF.Sigmoid)
                if reps == 1:
                    nc.sync.dma_start(out_d[:], o_sb[:])
                elif _rep == 0:
                    nc.vector.tensor_copy(oacc_sb[:], o_sb[:])
                else:
                    nc.vector.tensor_max(oacc_sb[:], oacc_sb[:], o_sb[:])
                    if _rep == reps - 1:
                        nc.sync.dma_start(out_d[:], oacc_sb[:])

            if cfg.get("defer_epilogue", False):
                pending_epi[0] = _epilogue
            else:
                _epilogue()
        if pending_epi[0] is not None:
            pending_epi[0]()
            pending_epi[0] = None

    nc.compile()
    return nc


def _get_state(cfg=None):
    global _STATE
    if _STATE is None:
        _STATE = _build_nc(cfg or CFG)
    return _STATE


def make_in_maps(videos, W_gcn, b_gcn, W1, b1, W2, b2, cfg=None):
    import ml_dtypes

    cfg = cfg or CFG
    gcn_np = ml_dtypes.float8_e4m3 if cfg["gcn_fp8"] else ml_dtypes.bfloat16
    bf16 = ml_dtypes.bfloat16

    videos = np.asarray(videos, dtype=np.float32)
    W_gcn = np.asarray(W_gcn, dtype=np.float32)
    W1 = np.asarray(W1, dtype=np.float32)
    W2 = np.asarray(W2, dtype=np.float32)

    # W_gcn [F, F] -> [128, KC*F]: wg[p, k*F+f] = W_gcn[k*128+p, f]
    wg_host = np.ascontiguousarray(
        W_gcn.reshape(KC, P, F).transpose(1, 0, 2).reshape(P, KC * F)
    ).astype(gcn_np)
    # W1 [V*F, H1] -> [128, 64*H1]: w1[p, i*H1+n] = W1[i*128+p, n]
    w1_host = np.ascontiguousarray(
        W1.reshape(V * KC, P, H1).transpose(1, 0, 2).reshape(P, V * KC * H1)
    ).astype(bf16)
    # W2 [2*128, OUT] -> [128, 2*OUT]
    w2_host = np.ascontiguousarray(
        W2.reshape(2, P, OUT).transpose(1, 0, 2).reshape(P, 2 * OUT)
    ).astype(bf16)
    bg_host = np.ascontiguousarray(
        np.asarray(b_gcn, np.float32).reshape(JC, P).T
    )
    b1_host = np.asarray(b1, np.float32).reshape(1, H1)
    b2_host = np.asarray(b2, np.float32).reshape(1, OUT)
    id8 = np.eye(BL, dtype=np.float32)

    common = {
        "wg": wg_host,
        "w1": w1_host,
        "w2": w2_host,
        "bg": bg_host,
        "b1": b1_host,
        "b2": b2_host,
        "id8": id8,
    }
    in_maps = []
    for i in range(NCORES):
        m = dict(common)
        # per-core X [BL, V, T, F] -> chunk-major X^T:
        # xt[c*128+p, k*CHUNK+n] = x[c, n, k*128+p]   (n = v*T+t)
        xc = videos[i * BL : (i + 1) * BL].reshape(NCH, CHUNK, KC, P)
        m["xt"] = np.ascontiguousarray(
            xc.transpose(0, 3, 2, 1).reshape(NCH * P, KC * CHUNK)
        ).astype(gcn_np)
        in_maps.append(m)
    return in_maps


_RUNNER = None


def _make_runner(nc):
    """Cached multi-core PJRT runner (mirrors bass2jax.run_bass_via_pjrt but
    jits once so repeated calls don't re-trace)."""
    import jax
    import numpy as _np
    from jax.experimental.shard_map import shard_map
    from jax.sharding import Mesh, PartitionSpec
    from concourse import bass2jax, mybir

    bass2jax.install_neuronx_cc_hook()
    assert nc.dbg_addr is None
    partition_name = (
        nc.partition_id_tensor.name if nc.partition_id_tensor is not None else None
    )

    in_names, out_names, out_avals, zero_outs = [], [], [], []
    for alloc in nc.m.functions[0].allocations:
        if not isinstance(alloc, mybir.MemoryLocationSet):
            continue
        name = alloc.memorylocations[0].name
        if alloc.kind == "ExternalInput":
            if name != partition_name:
                in_names.append(name)
        elif alloc.kind == "ExternalOutput":
            out_names.append(name)
            shape = tuple(alloc.tensor_shape)
            dtype = mybir.dt.np(alloc.dtype)
            out_avals.append(jax.core.ShapedArray(shape, dtype))
            zero_outs.append(_np.zeros(shape, dtype))
    n_params = len(in_names)
    n_outs = len(out_avals)
    all_names = in_names + out_names
    if partition_name is not None:
        all_names = all_names + [partition_name]

    def _body(*args):
        operands = list(args)
        if partition_name is not None:
            operands.append(bass2jax.partition_id_tensor())
        outs = bass2jax._bass_exec_p.bind(
            *operands,
            out_avals=tuple(out_avals),
            in_names=tuple(all_names),
            out_names=tuple(out_names),
            lowering_input_output_aliases=(),
            sim_require_finite=True,
            sim_require_nnan=True,
            nc=nc,
        )
        return tuple(outs)

    devices = jax.devices()[:NCORES]
    mesh = Mesh(np.asarray(devices), ("core",))
    in_specs = (PartitionSpec("core"),) * (n_params + n_outs)
    out_specs = (PartitionSpec("core"),) * n_outs
    sharded = jax.jit(
        shard_map(
            _body, mesh=mesh, in_specs=in_specs, out_specs=out_specs, check_rep=False
        ),
        keep_unused=True,
    )

    def run(in_maps, device_inputs=None):
        if device_inputs is None:
            device_inputs = prep(in_maps)
        out_arrs = sharded(*device_inputs)
        jax.block_until_ready(out_arrs)
        return [
            {
                name: _np.asarray(out_arrs[i]).reshape(NCORES, *out_avals[i].shape)[c]
                for i, name in enumerate(out_names)
            }
            for c in range(NCORES)
        ]

    def prep(in_maps):
        from jax.sharding import NamedSharding

        concat_in = [
            _np.concatenate([_np.asarray(in_maps[c][nm]) for c in range(NCORES)], 0)
            for nm in in_names
        ]
        concat_zeros = [
            _np.zeros((NCORES * z.shape[0], *z.shape[1:]), z.dtype) for z in zero_outs
        ]
        sh = NamedSharding(mesh, PartitionSpec("core"))
        arrs = [jax.device_put(a, sh) for a in concat_in + concat_zeros]
        jax.block_until_ready(arrs)
        return arrs

    return run, prep


def _get_runner():
    global _RUNNER
    if _RUNNER is None:
        _RUNNER = _make_runner(_get_state())
    return _RUNNER


def run_spmd(in_maps, device_inputs=None):
    run, _ = _get_runner()
    return run(in_maps, device_inputs)


def prep_inputs(in_maps):
    _, prep = _get_runner()
    return prep(in_maps)


def kernel(videos, W_gcn, b_gcn, W1, b1, W2, b2):
    in_maps = make_in_maps(videos, W_gcn, b_gcn, W1, b1, W2, b2)
    results = run_spmd(in_maps)
    out = np.stack([results[i]["out"] for i in range(NCORES)])  # [8, 8, 512]
    return out.reshape(B, OUT).reshape(B, V, T).astype(np.float32)


# revision 50
# speedup vs baseline: 1.0021x; 1.0021x over previous
"""Trainium2 Bass kernel for KeyframeSelectionNetwork.

Math (per (b, v) video of T=64 frames, F=1024 features):
  GCN with self-loops + one edge (frame0 -> frame1), symmetric norm:
    out[t] = x[t] @ W_gcn                      for t != 1
    out[1] = (0.5*x[1] + (1/sqrt(2))*x[0]) @ W_gcn
  pooled = max_t out[t] + b_gcn
  h = relu(pooled.reshape(B, V*F) @ W1 + b1)  -> [B, 256]
  key = sigmoid(h @ W2 + b2)                  -> [B, V, T]

Strategy: data-parallel over batch across 8 cores (8 videos' batches each).
Host-side prep (per core): X is pre-transposed to chunk-major X^T layout
([chunk, 128 f-part, KC, 512 nodes]) and cast to fp8-e4m3; W_gcn to fp8,
W1/W2 to bf16, all pre-arranged so every load is a single fat contiguous
HWDGE DMA (per-partition lines of 4-32KB).

Device per rep (~9.3 MB HBM traffic, ~70us, PE-bound at the fp8 roofline):
  - ~21 fat DMAs alternating the two HWDGE queues (sync/act); X chunks in
    k-halves so matmuls start on first-half arrival.
  - edge combine applied in-place on the fp8 X tile (DVE ops per k-half on
    [128, KC/2, 8] strided views: x1' = 0.5*x1 + rsqrt2*x0).
  - GCN matmul Y^T[fout, nodes] via fp8 DoubleRow PE matmuls (contracts
    2x128 K-rows per pass, 2x bf16 throughput), PSUM-accumulated.
  - max-pool over t via DVE reduce_max per PSUM tile -> pooledT [fout, b*v],
    then bias-add + bf16 cast per (j, chunk) on the idle gpsimd engine.
  - MLP in bf16: pooled slices as stationary, biases folded in as rank-1
    matmuls (ones.T @ b), relu/sigmoid on ACT engine.
  - PSUM pools are shared across reps (no per-rep pool barriers) and the
    per-rep outputs are max-combined into one [BL, OUT] store so the
    R-reps timing NEFF has the same host-visible output as R=1.
"""

import sys

sys.path.insert(0, "/opt/trn_rl_repo")

import numpy as np

B, V, T, F = 64, 8, 64, 1024
NCORES = 8
BL = B // NCORES  # batches per core (8)
NLOC = BL * V * T  # nodes per core (4096)
H1 = 256
OUT = V * T  # 512
P = 128
KC = F // P  # 8 contraction chunks
JC = F // P  # 8 output-feature chunks
CHUNK = V * T  # 512 nodes per chunk = one batch's videos
NCH = NLOC // CHUNK  # 8 chunks per core
RT2 = float(1.0 / np.sqrt(2.0))

CFG = dict(
    gcn_fp8=True,  # fp8 X/W_gcn with DoubleRow matmuls; else bf16
    combine_on_x=True,  # edge combine on X tile (else post-matmul in PSUM)
    x_bufs=8,
    w_bufs=2,
    psum_bufs=7,  # 7 x 1-bank yp tiles + 1 shared epilogue bank
    defer_epilogue=False,  # emitting MLP mid-next-rep measured slightly worse
    fused_bias_cast=True,  # bias+bf16-cast per (j,c) right after each reduce
)

_STATE = None


def _build_nc(cfg, reps=1):
    import concourse.bacc as bacc
    import concourse.tile as tile
    from concourse import mybir

    f32 = mybir.dt.float32
    bf16 = mybir.dt.bfloat16
    fp8 = mybir.dt.float8e4
    AF = mybir.ActivationFunctionType
    ALU = mybir.AluOpType
    gcn_dt = fp8 if cfg["gcn_fp8"] else bf16

    nc = bacc.Bacc(None, target_bir_lowering=False, debug=False)
    x_d = nc.dram_tensor("xt", [NCH * P, KC * CHUNK], gcn_dt, kind="ExternalInput")
    wg_d = nc.dram_tensor("wg", [P, KC * F], gcn_dt, kind="ExternalInput")
    w1_d = nc.dram_tensor("w1", [P, V * KC * H1], bf16, kind="ExternalInput")
    w2_d = nc.dram_tensor("w2", [P, 2 * OUT], bf16, kind="ExternalInput")
    bg_d = nc.dram_tensor("bg", [P, JC], f32, kind="ExternalInput")
    b1_d = nc.dram_tensor("b1", [1, H1], f32, kind="ExternalInput")
    b2_d = nc.dram_tensor("b2", [1, OUT], f32, kind="ExternalInput")
    id8_d = nc.dram_tensor("id8", [BL, BL], f32, kind="ExternalInput")
    # Single [BL, OUT] output for any reps: per-rep results are max-combined
    # on device (keeps every rep live against DCE) so the host-side output
    # fetch is identical for the R=1 and R=reps NEFFs and cancels in the
    # marginal-time measurement.
    out_d = nc.dram_tensor("out", [BL, OUT], f32, kind="ExternalOutput")

    with tile.TileContext(nc) as tc:
      with (
          tc.tile_pool(name="xpool", bufs=cfg["x_bufs"]) as xpool,
          tc.tile_pool(name="wpool", bufs=cfg["w_bufs"]) as wpool,
          tc.tile_pool(name="spool", bufs=2) as spool,
          tc.tile_pool(name="fpool", bufs=2) as fpool,
          tc.tile_pool(name="apool", bufs=1) as apool,
          tc.tile_pool(name="mpsum", bufs=cfg["psum_bufs"], space="PSUM") as mpsum,
          tc.tile_pool(name="lpsum", bufs=1, space="PSUM") as lpsum,
      ):
        oacc_sb = None
        if reps > 1:
            oacc_sb = apool.tile([BL, OUT], f32, tag="oacc", name="oacc_sb")
        # Software pipelining across reps: each rep's epilogue (MLP) is
        # emitted AFTER the next rep's first chunk of Y-matmuls, so the PE's
        # in-order queue never stalls waiting for the pooling reduces.
        pending_epi = [None]
        for _rep in range(reps):
            # ---- prologue DMAs (all fat contiguous loads) ----
            wg_sb = wpool.tile([P, KC, F], gcn_dt, tag="wg")
            nc.sync.dma_start(
                wg_sb[:], wg_d[:].rearrange("p (k f) -> p k f", f=F)
            )
            xts = {}

            def load_x(c):
                t_ = xpool.tile([P, KC, CHUNK], gcn_dt, tag="x")
                # alternate the two HWDGE queues (sync/act) for DMA
                # throughput; split in k-halves so the first matmuls of the
                # chunk only wait on the first half's arrival
                eng = nc.sync if c % 2 == 0 else nc.scalar
                src = x_d[c * P : (c + 1) * P, :].rearrange(
                    "p (k n) -> p k n", n=CHUNK
                )
                half = KC // 2
                eng.dma_start(t_[:, :half, :], src[:, :half, :])
                eng.dma_start(t_[:, half:, :], src[:, half:, :])
                xts[c] = t_

            nx_pre = min(cfg["x_bufs"], NCH)
            for c in range(nx_pre):
                load_x(c)
            bg_sb = spool.tile([P, JC], f32, tag="bg")
            nc.sync.dma_start(bg_sb[:], bg_d[:])
            b1_sb = spool.tile([1, H1], f32, tag="b1")
            nc.sync.dma_start(b1_sb[:], b1_d[:])
            b2_sb = spool.tile([1, OUT], f32, tag="b2")
            nc.sync.dma_start(b2_sb[:], b2_d[:])
            id8_sb = spool.tile([BL, BL], f32, tag="id8")
            nc.sync.dma_start(id8_sb[:], id8_d[:])
            ones_sb = spool.tile([1, BL], f32, tag="ones")
            nc.gpsimd.memset(ones_sb[:], 1.0)
            w2_sb = wpool.tile([P, 2, OUT], bf16, tag="w2")
            nc.sync.dma_start(
                w2_sb[:], w2_d[:].rearrange("p (m n) -> p m n", n=OUT)
            )
            # w1 (4MB) is loaded in 8 x 512KB pieces interleaved with the
            # chunk loop (see below): a monolithic DMA here would sit on the
            # scalar queue ahead of the NEXT rep's odd-chunk X loads, which
            # are needed ~7us into the rep while w1 isn't needed until the
            # epilogue (priority inversion seen as early-rep PE gaps).
            w1_sb = wpool.tile([P, V * KC, H1], bf16, tag="w1")
            w1_src = w1_d[:].rearrange("p (i n) -> p i n", n=H1)

            pooledT = spool.tile([P, JC, NCH * V], f32, tag="pooledT")
            pooled_m = spool.tile([P, JC, NCH * V], bf16, tag="pooled_m")

            # ---- main loop: per chunk, combine fixup + JC matmul/pool ----
            if True:
                for c in range(NCH):
                    xt = xts.pop(c)
                    if cfg["combine_on_x"]:
                        # x1' = 0.5*x1 + rsqrt2*x0, in place on the fp8 tile
                        # (done per k-half to match the split DMA arrival)
                        x4 = xt[:].rearrange("p k (g t) -> p k g t", t=T)
                        for h in range(2):
                            ks = slice(h * KC // 2, (h + 1) * KC // 2)
                            col0 = x4[:, ks, :, 0]
                            col1 = x4[:, ks, :, 1]
                            nc.vector.tensor_scalar_mul(col1, col1, 0.5)
                            nc.vector.scalar_tensor_tensor(
                                col1, col0, RT2, col1, ALU.mult, ALU.add
                            )
                    for j in range(JC):
                        yp = mpsum.tile([P, CHUNK], f32, tag="yp")
                        for q in range(KC // 2):
                            nc.tensor.matmul(
                                yp[:],
                                wg_sb[:, 2 * q : 2 * q + 2, j * P : (j + 1) * P],
                                xt[:, 2 * q : 2 * q + 2, :],
                                start=(q == 0),
                                stop=(q == KC // 2 - 1),
                                perf_mode=mybir.MatmulPerfMode.DoubleRow,
                            )
                        nc.vector.reduce_max(
                            pooledT[:, j, c * V : (c + 1) * V],
                            yp[:].rearrange("p (g t) -> p g t", t=T),
                            axis=mybir.AxisListType.X,
                        )
                        if cfg["fused_bias_cast"]:
                            # bias-add + bf16 cast on the idle gpsimd engine
                            nc.gpsimd.tensor_scalar_add(
                                pooled_m[:, j, c * V : (c + 1) * V],
                                pooledT[:, j, c * V : (c + 1) * V],
                                bg_sb[:, j : j + 1],
                            )
                    if c == 0 and pending_epi[0] is not None:
                        pending_epi[0]()
                        pending_epi[0] = None
                    nc.scalar.dma_start(
                        w1_sb[:, c * V : (c + 1) * V, :],
                        w1_src[:, c * V : (c + 1) * V, :],
                    )
                    if c + nx_pre < NCH:
                        load_x(c + nx_pre)

            # ---- epilogue: bias (fused into bf16 cast), MLP; deferred
            # until after the next rep's first chunk (see pending_epi) ----
            def _epilogue(
                _rep=_rep,
                pooledT=pooledT,
                pooled_m=pooled_m,
                bg_sb=bg_sb,
                w1_sb=w1_sb,
                w2_sb=w2_sb,
                b1_sb=b1_sb,
                b2_sb=b2_sb,
                id8_sb=id8_sb,
                ones_sb=ones_sb,
            ):
                if not cfg["fused_bias_cast"]:
                    for j in range(JC):
                        nc.scalar.activation(
                            pooled_m[:, j, :],
                            pooledT[:, j, :],
                            AF.Identity,
                            bias=bg_sb[:, j : j + 1],
                        )

                # hp/thp/op share one lpsum slot (used strictly sequentially,
                # same tag) so the epilogue needs only 1 PSUM bank
                hp = lpsum.tile([BL, H1], f32, tag="lp", name="hp")
                for v in range(V):
                    for fc in range(KC):
                        i = v * KC + fc
                        lhs = pooled_m[:, fc, :].rearrange(
                            "p (b w) -> p w b", w=V
                        )[:, v, :]
                        nc.tensor.matmul(
                            hp[:], lhs, w1_sb[:, i, :], start=(i == 0), stop=False
                        )
                nc.tensor.matmul(hp[:], ones_sb[:], b1_sb[:], start=False, stop=True)
                h_sb = spool.tile([BL, H1], f32, tag="h")
                nc.scalar.activation(h_sb[:], hp[:], AF.Relu)

                ht_sb = spool.tile([P, 2, BL], bf16, tag="ht")
                for m in range(2):
                    thp = lpsum.tile([P, BL], f32, tag="lp", name="thp")
                    nc.tensor.transpose(
                        thp[:], h_sb[:, m * P : (m + 1) * P], id8_sb[:]
                    )
                    nc.vector.tensor_copy(ht_sb[:, m, :], thp[:])

                op = lpsum.tile([BL, OUT], f32, tag="lp", name="op")
                for m in range(2):
                    nc.tensor.matmul(
                        op[:], ht_sb[:, m, :], w2_sb[:, m, :], start=(m == 0),
                        stop=False,
                    )
                nc.tensor.matmul(op[:], ones_sb[:], b2_sb[:], start=False, stop=True)
                o_sb = spool.tile([BL, OUT], f32, tag="o")
                nc.scalar.activation(o_sb[:], op[:], AF.Sigmoid)
                if reps == 1:
                    nc.sync.dma_start(out_d[:], o_sb[:])
                elif _rep == 0:
                    nc.vector.tensor_copy(oacc_sb[:], o_sb[:])
                else:
                    nc.vector.tensor_max(oacc_sb[:], oacc_sb[:], o_sb[:])
                    if _rep == reps - 1:
                        nc.sync.dma_start(out_d[:], oacc_sb[:])

            if cfg.get("defer_epilogue", False):
                pending_epi[0] = _epilogue
            else:
                _epilogue()
        if pending_epi[0] is not None:
            pending_epi[0]()
            pending_epi[0] = None

    nc.compile()
    return nc


def _get_state(cfg=None):
    global _STATE
    if _STATE is None:
        _STATE = _build_nc(cfg or CFG)
    return _STATE


def make_in_maps(videos, W_gcn, b_gcn, W1, b1, W2, b2, cfg=None):
    import ml_dtypes

    cfg = cfg or CFG
    gcn_np = ml_dtypes.float8_e4m3 if cfg["gcn_fp8"] else ml_dtypes.bfloat16
    bf16 = ml_dtypes.bfloat16

    videos = np.asarray(videos, dtype=np.float32)
    W_gcn = np.asarray(W_gcn, dtype=np.float32)
    W1 = np.asarray(W1, dtype=np.float32)
    W2 = np.asarray(W2, dtype=np.float32)

    # W_gcn [F, F] -> [128, KC*F]: wg[p, k*F+f] = W_gcn[k*128+p, f]
    wg_host = np.ascontiguousarray(
        W_gcn.reshape(KC, P, F).transpose(1, 0, 2).reshape(P, KC * F)
    ).astype(gcn_np)
    # W1 [V*F, H1] -> [128, 64*H1]: w1[p, i*H1+n] = W1[i*128+p, n]
    w1_host = np.ascontiguousarray(
        W1.reshape(V * KC, P, H1).transpose(1, 0, 2).reshape(P, V * KC * H1)
    ).astype(bf16)
    # W2 [2*128, OUT] -> [128, 2*OUT]
    w2_host = np.ascontiguousarray(
        W2.reshape(2, P, OUT).transpose(1, 0, 2).reshape(P, 2 * OUT)
    ).astype(bf16)
    bg_host = np.ascontiguousarray(
        np.asarray(b_gcn, np.float32).reshape(JC, P).T
    )
    b1_host = np.asarray(b1, np.float32).reshape(1, H1)
    b2_host = np.asarray(b2, np.float32).reshape(1, OUT)
    id8 = np.eye(BL, dtype=np.float32)

    common = {
        "wg": wg_host,
        "w1": w1_host,
        "w2": w2_host,
        "bg": bg_host,
        "b1": b1_host,
        "b2": b2_host,
        "id8": id8,
    }
    in_maps = []
    for i in range(NCORES):
        m = dict(common)
        # per-core X [BL, V, T, F] -> chunk-major X^T:
        # xt[c*128+p, k*CHUNK+n] = x[c, n, k*128+p]   (n = v*T+t)
        xc = videos[i * BL : (i + 1) * BL].reshape(NCH, CHUNK, KC, P)
        m["xt"] = np.ascontiguousarray(
            xc.transpose(0, 3, 2, 1).reshape(NCH * P, KC * CHUNK)
        ).astype(gcn_np)
        in_maps.append(m)
    return in_maps


_RUNNER = None


def _make_runner(nc):
    """Cached multi-core PJRT runner (mirrors bass2jax.run_bass_via_pjrt but
    jits once so repeated calls don't re-trace)."""
    import jax
    import numpy as _np
    from jax.experimental.shard_map import shard_map
    from jax.sharding import Mesh, PartitionSpec
    from concourse import bass2jax, mybir

    bass2jax.install_neuronx_cc_hook()
    assert nc.dbg_addr is None
    partition_name = (
        nc.partition_id_tensor.name if nc.partition_id_tensor is not None else None
    )

    in_names, out_names, out_avals, zero_outs = [], [], [], []
    for alloc in nc.m.functions[0].allocations:
        if not isinstance(alloc, mybir.MemoryLocationSet):
            continue
        name = alloc.memorylocations[0].name
        if alloc.kind == "ExternalInput":
            if name != partition_name:
                in_names.append(name)
        elif alloc.kind == "ExternalOutput":
            out_names.append(name)
            shape = tuple(alloc.tensor_shape)
            dtype = mybir.dt.np(alloc.dtype)
            out_avals.append(jax.core.ShapedArray(shape, dtype))
            zero_outs.append(_np.zeros(shape, dtype))
    n_params = len(in_names)
    n_outs = len(out_avals)
    all_names = in_names + out_names
    if partition_name is not None:
        all_names = all_names + [partition_name]

    def _body(*args):
        operands = list(args)
        if partition_name is not None:
            operands.append(bass2jax.partition_id_tensor())
        outs = bass2jax._bass_exec_p.bind(
            *operands,
            out_avals=tuple(out_avals),
            in_names=tuple(all_names),
            out_names=tuple(out_names),
            lowering_input_output_aliases=(),
            sim_require_finite=True,
            sim_require_nnan=True,
            nc=nc,
        )
        return tuple(outs)

    devices = jax.devices()[:NCORES]
    mesh = Mesh(np.asarray(devices), ("core",))
    in_specs = (PartitionSpec("core"),) * (n_params + n_outs)
    out_specs = (PartitionSpec("core"),) * n_outs
    sharded = jax.jit(
        shard_map(
            _body, mesh=mesh, in_specs=in_specs, out_specs=out_specs, check_rep=False
        ),
        keep_unused=True,
    )

    def run(in_maps, device_inputs=None):
        if device_inputs is None:
            device_inputs = prep(in_maps)
        out_arrs = sharded(*device_inputs)
        jax.block_until_ready(out_arrs)
        return [
            {
                name: _np.asarray(out_arrs[i]).reshape(NCORES, *out_avals[i].shape)[c]
                for i, name in enumerate(out_names)
            }
            for c in range(NCORES)
        ]

    def prep(in_maps):
        from jax.sharding import NamedSharding

        concat_in = [
            _np.concatenate([_np.asarray(in_maps[c][nm]) for c in range(NCORES)], 0)
            for nm in in_names
        ]
        concat_zeros = [
            _np.zeros((NCORES * z.shape[0], *z.shape[1:]), z.dtype) for z in zero_outs
        ]
        sh = NamedSharding(mesh, PartitionSpec("core"))
        arrs = [jax.device_put(a, sh) for a in concat_in + concat_zeros]
        jax.block_until_ready(arrs)
        return arrs

    return run, prep


def _get_runner():
    global _RUNNER
    if _RUNNER is None:
        _RUNNER = _make_runner(_get_state())
    return _RUNNER


def run_spmd(in_maps, device_inputs=None):
    run, _ = _get_runner()
    return run(in_maps, device_inputs)


def prep_inputs(in_maps):
    _, prep = _get_runner()
    return prep(in_maps)


def kernel(videos, W_gcn, b_gcn, W1, b1, W2, b2):
    in_maps = make_in_maps(videos, W_gcn, b_gcn, W1, b1, W2, b2)
    results = run_spmd(in_maps)
    out = np.stack([results[i]["out"] for i in range(NCORES)])  # [8, 8, 512]
    return out.reshape(B, OUT).reshape(B, V, T).astype(np.float32)
